# revision 29
# baseline (speedup 1.0000x reference)
"""GAT (3-layer, PyG GATConv-style) Trainium2 Bass kernel, 8-core SPMD.

Strategy (degree-bucketed dst-major fixed-degree layout):
  - Nodes are permuted by in-degree (desc) and assigned to (core, block,
    partition): chunk b of 1024 sorted nodes -> block b on every core.
    Per-block slot count D_b = max in-degree within the chunk (padded to a
    multiple of 8), so padding waste stays ~15%.
  - Per layer: each core computes h_aug = x @ W_aug for its 2560 nodes
    (W_aug fuses per-head a_src/a_dst projections as trailing columns),
    stores to DRAM, one AllGather -> full node table hf.
  - Phase C per block of 128 dst nodes: dma_gather pulls the D_b incident
    src rows per dst into [128 dst, D_b, RW] (slot-major index tables), then
    a handful of giant DVE ops do the whole block: p = exp(leakyrelu(
    s_src + s_dst)), numer = reduce_d(p * h), denom = reduce_d(p),
    out = head_mean(numer / denom).  Padding slots point at a poisoned row
    (s_src = -1e9 -> p = 0), so no masking is needed.
  - Layer boundary: out blocks stored node-major to DRAM; the next layer's
    transposed activations are re-loaded via dma_gather(transpose=True)
    with an identity index table.
  - Layer 3 ends with a ones-vector matmul accumulating the node-sum
    partial; host sums the 8 per-core [1,128] partials.
"""

import numpy as np
import ml_dtypes

BF16 = ml_dtypes.bfloat16
NCORES = 8
GC = 6  # slots per gather chunk (6*128 = 768 idxs = 48 desc/engine)


# ----------------------------------------------------------------------------
# Host-side preprocessing
# ----------------------------------------------------------------------------

def _wrap16(idx_flat):
    """dma_gather index layout: [128, n/16] int16, idx i at [i%16, i//16],
    replicated across the 8 groups of 16 partitions."""
    n = idx_flat.shape[0]
    assert n % 16 == 0
    w = idx_flat.reshape(n // 16, 16).T.astype(np.int16)  # [16, n/16]
    return np.tile(w, (8, 1))  # [128, n/16]


def _wcols(H, C):
    """useful h_aug columns: H*(C+1) features+ones + 2H scores."""
    return H * (C + 1) + 2 * H


def _row_bytes(H, C):
    """gathered row bytes: H*(C+1) fp8 features+ones, then 2H bf16 scores at
    the next even byte, padded to a 256B multiple (dma_gather constraint)."""
    sbo = ((H * (C + 1) + 1) // 2) * 2
    used = sbo + 4 * H
    return ((used + 255) // 256) * 256


def _wpad(H, C):
    """W_aug padded column count (psum tile geometry: 512+128 or 256)."""
    return 640 if _wcols(H, C) > 512 else 256


def prep_static(edge_index, N, NPAD):
    """Degree-sorted node permutation + slot-major gather tables.

    Returns (Dpad, idx_cores, node_of_row, PAD_P0)."""
    loops = np.arange(N, dtype=np.int64)
    src = np.concatenate([edge_index[0].astype(np.int64), loops])
    dst = np.concatenate([edge_index[1].astype(np.int64), loops])
    deg = np.bincount(dst, minlength=NPAD)  # pad nodes have degree 0
    order = np.argsort(-deg, kind="stable")

    BPC = NPAD // (128 * NCORES)
    NPC = NPAD // NCORES
    node_of_row = np.empty(NPAD, dtype=np.int64)
    for b in range(BPC - 1):
        chunk = order[b * 1024:(b + 1) * 1024]
        q = np.arange(1024)
        rows = (q // 128) * NPC + b * 128 + (q % 128)
        node_of_row[rows] = chunk
    # last chunk: reals first on every core, pads fill the tail partitions
    last = order[(BPC - 1) * 1024:]
    n_real = int((deg[last] > 0).sum())
    assert n_real % NCORES == 0
    reals, pads = last[:n_real], last[n_real:]
    rpc = n_real // NCORES
    ppc = (1024 - n_real) // NCORES
    b = BPC - 1
    for c in range(NCORES):
        base = c * NPC + b * 128
        node_of_row[base:base + rpc] = reals[c * rpc:(c + 1) * rpc]
        node_of_row[base + rpc:base + 128] = pads[c * ppc:(c + 1) * ppc]
    row_of_node = np.empty(NPAD, dtype=np.int64)
    row_of_node[node_of_row] = np.arange(NPAD)
    PAD_P0 = rpc
    PADROW = NPAD  # dedicated poison row appended past the node table

    Dpad = []
    for b in range(BPC):
        mx = int(deg[order[b * 1024]])
        Dpad.append(max(1, mx))

    sidx = np.argsort(dst, kind="stable")
    src_s, dst_s = src[sidx], dst[sidx]
    starts = np.searchsorted(dst_s, np.arange(NPAD))
    ends = np.searchsorted(dst_s, np.arange(NPAD) + 1)

    idx_cores = []
    for c in range(NCORES):
        cols = []
        for b in range(BPC):
            D = Dpad[b]
            flat = np.full(D * 128, PADROW, dtype=np.int64)
            for p in range(128):
                v = node_of_row[c * NPC + b * 128 + p]
                s0, s1 = starts[v], ends[v]
                k = s1 - s0
                if k:
                    flat[np.arange(k) * 128 + p] = row_of_node[src_s[s0:s1]]
            cols.append(_wrap16(flat))
        idx_cores.append(np.ascontiguousarray(np.concatenate(cols, axis=1)))
    return tuple(Dpad), idx_cores, node_of_row, PAD_P0


def prep_values(x, Ws, a_srcs, a_dsts, NPAD, node_of_row):
    N, F = x.shape
    xp = np.zeros((NPAD, F), dtype=np.float32)
    xp[:N] = x
    xperm = xp[node_of_row]  # row r holds node node_of_row[r]
    xT = np.ascontiguousarray(xperm.T).astype(BF16)  # [F, NPAD]

    W_augs = []
    for W, a_s, a_d in zip(Ws, a_srcs, a_dsts):
        H, Fin, C = W.shape
        FW = H * (C + 1)
        wsrc = np.einsum("hfc,hc->fh", W, a_s)
        wdst = np.einsum("hfc,hc->fh", W, a_d)
        Wa = np.zeros((Fin, _wpad(H, C)), dtype=np.float32)
        for h in range(H):
            # col h*(C+1)+C stays 0: the ones column, memset on device
            Wa[:, h * (C + 1):h * (C + 1) + C] = W[h].reshape(Fin, C)
        Wa[:, FW:FW + H] = wsrc
        Wa[:, FW + H:FW + 2 * H] = wdst
        W_augs.append(Wa.astype(BF16))
    return xT, W_augs


# ----------------------------------------------------------------------------
# Device program
# ----------------------------------------------------------------------------

def build_nc(cfg, repeat=1):
    import concourse.bacc as bacc
    import concourse.mybir as mybir
    import concourse.tile as tile
    from contextlib import ExitStack

    f32 = mybir.dt.float32
    bf16 = mybir.dt.bfloat16
    f8 = mybir.dt.float8e4
    i16 = mybir.dt.int16
    ALU = mybir.AluOpType
    ACT = mybir.ActivationFunctionType
    AX = mybir.AxisListType

    N = cfg["N"]
    NPAD = cfg["NPAD"]
    F_IN = cfg["F_IN"]
    C = cfg["C"]
    Dpad = cfg["Dpad"]
    HS = cfg["HS"]
    PAD_P0 = cfg["PAD_P0"]
    BPC = NPAD // (128 * NCORES)
    NPC = NPAD // NCORES
    NL = len(HS)
    RWBs = [_row_bytes(H, C) for H in HS]
    WPs = [_wpad(H, C) for H in HS]
    FINs = [F_IN] + [C] * (NL - 1)
    DSUM = sum(Dpad)
    doff = [0]
    for d in Dpad:
        doff.append(doff[-1] + d)
    Dmax = max(Dpad)
    G1W = Dmax * max(RWBs)

    nc = bacc.Bacc("TRN2", target_bir_lowering=False, debug=False,
                   num_devices=NCORES)

    xT_d = nc.dram_tensor("xT", [F_IN, NPC], bf16, kind="ExternalInput")
    idx_d = nc.dram_tensor("idxs", [128, DSUM * 8], i16, kind="ExternalInput")
    W_d = [nc.dram_tensor(f"w{i+1}", [FINs[i], WPs[i]], bf16,
                          kind="ExternalInput") for i in range(NL)]
    bb_d = [nc.dram_tensor(f"bb{i+1}", [128, C], f32, kind="ExternalInput")
            for i in range(NL - 1)]
    out_d = nc.dram_tensor("out", [1, C], f32, kind="ExternalOutput")

    with tile.TileContext(nc, num_cores=NCORES) as tc, ExitStack() as ctx:
        dram = ctx.enter_context(tc.tile_pool(name="dram", bufs=1, space="DRAM"))
        cpool = ctx.enter_context(tc.tile_pool(name="consts", bufs=1))
        hpool = ctx.enter_context(tc.tile_pool(name="hs", bufs=1))
        wpool = ctx.enter_context(tc.tile_pool(name="work", bufs=1))
        psum = ctx.enter_context(tc.tile_pool(name="ps", bufs=2, space="PSUM"))

        hl = [dram.tile([NPC, RWBs[i]], f8, tag=f"hl{i}", name=f"hl{i}")
              for i in range(NL)]
        # one extra row past the node table: the poison row pad slots point at
        hf = [dram.tile([NPAD + 1, RWBs[i]], f8, tag=f"hf{i}", name=f"hf{i}")
              for i in range(NL)]
        x2d = [dram.tile([NPC, C], bf16, tag=f"x2d{i}", name=f"x2d{i}")
               for i in range(NL - 1)]

        xT_sb = cpool.tile([F_IN, NPC], bf16, tag="xT")
        nc.sync.dma_start(xT_sb[:], xT_d[:, :])
        idx_sb = cpool.tile([128, DSUM * 8], i16, tag="idx")
        nc.sync.dma_start(idx_sb[:], idx_d[:, :])
        W_sb = []
        for i in range(NL):
            w = cpool.tile([FINs[i], WPs[i]], bf16, tag=f"w{i}", name=f"w{i}")
            nc.sync.dma_start(w[:], W_d[i][:, :])
            W_sb.append(w)
        bb_sb = []
        for i in range(NL - 1):
            t = cpool.tile([128, C], f32, tag=f"bb{i}", name=f"bb{i}")
            nc.sync.dma_start(t[:], bb_d[i][:, :])
            bb_sb.append(t)
        ones_sb = cpool.tile([128, 1], f32, tag="ones")
        nc.vector.memset(ones_sb[:], 1.0)
        pois_sb = cpool.tile([1, max(RWBs)], f8, tag="pois")
        nc.vector.memset(pois_sb[:].bitcast(bf16), -1e9)
        x2T = [cpool.tile([C, NPC], bf16, tag=f"x2T{i}", name=f"x2T{i}")
               for i in range(NL - 1)]
        g1f = cpool.tile([128, G1W], f8, tag="g1f")
        hs4c = cpool.tile([128, 4, max(RWBs)], f8, tag="hs4c")
        nc.vector.memset(hs4c[:], 0.0)
        msgf = cpool.tile([128, max(HS) * (C + 1) * Dmax], bf16, tag="msgf")

        pfin = psum.tile([1, C], f32, tag="pfin", bufs=1)
        dreg = {d: nc.gpsimd.to_reg(d * 128) for d in sorted(set(Dpad))}

        for _rep in range(repeat):
         for L in range(NL):
            H = HS[L]
            RWB = RWBs[L]
            FW = H * (C + 1)
            SBO = ((FW + 1) // 2) * 2      # scores byte offset
            SB2 = SBO // 2                 # ... in bf16 elems
            WCOL = FW + 2 * H

            # ---- phase A: h_aug for own nodes ----
            if L > 0:
                # transposed activations via DMA XBAR transpose
                nc.sync.dma_start(x2T[L - 1][:, :], x2d[L - 1][:, :],
                                  transpose=True)
            lsrc = xT_sb if L == 0 else x2T[L - 1]
            WP = WPs[L]
            for g0 in range(0, BPC, 4):
                hs4 = hs4c[:, :, 0:RWB]
                hs4s = hs4.bitcast(bf16)  # [128, 4, RWB//2]
                if WP > 512:
                    pA4 = psum.tile([128, 4, 512], f32, tag="pA4", bufs=1)
                    pB4 = psum.tile([128, 4, WP - 512], f32, tag="pB4", bufs=1)
                else:
                    pA4 = psum.tile([128, 4, WP], f32, tag="pA4", bufs=1)
                for j in range(4):
                    lhsT = lsrc[:, (g0 + j) * 128:(g0 + j + 1) * 128]
                    if WP > 512:
                        nc.tensor.matmul(pA4[:, j, :], lhsT, W_sb[L][:, 0:512],
                                         start=True, stop=True)
                        nc.tensor.matmul(pB4[:, j, :], lhsT, W_sb[L][:, 512:WP],
                                         start=True, stop=True)
                    else:
                        nc.tensor.matmul(pA4[:, j, :], lhsT, W_sb[L][:, 0:WP],
                                         start=True, stop=True)
                if WP > 512:
                    nc.scalar.copy(hs4[:, :, 0:512], pA4[:])
                    nc.vector.tensor_copy(hs4[:, :, 512:FW],
                                          pB4[:, :, 0:FW - 512])
                    nc.vector.tensor_copy(hs4s[:, :, SB2:SB2 + 2 * H],
                                          pB4[:, :, FW - 512:WCOL - 512])
                else:
                    nc.scalar.copy(hs4[:, :, 0:FW], pA4[:, :, 0:FW])
                    nc.vector.tensor_copy(hs4s[:, :, SB2:SB2 + 2 * H],
                                          pA4[:, :, FW:WCOL])
                ones_ap = hs4[:, :, 0:FW].rearrange(
                    "p j (h c) -> p j h c", c=C + 1)[:, :, :, C]
                nc.vector.memset(ones_ap, 1.0)
                dst_ap = hl[L][g0 * 128:(g0 + 4) * 128, :].rearrange(
                    "(j p) w -> p j w", j=4)
                nc.sync.dma_start(dst_ap, hs4)

            # ---- phase B: allgather + poison row ----
            nc.gpsimd.collective_compute(
                "AllGather", mybir.AluOpType.bypass,
                replica_groups=[list(range(NCORES))],
                ins=[hl[L][:, :].opt()],
                outs=[hf[L][0:NPAD, :].opt()],
            )
            nc.sync.dma_start(hf[L][NPAD:NPAD + 1, :], pois_sb[:, 0:RWB])
            # own nodes' s_dst, [128, BPC, H]
            sdl = wpool.tile([128, BPC, H], bf16, tag="sdl")
            hl_ap = hl[L][:, :].bitcast(bf16).rearrange(
                "(b p) w -> p b w", p=128)
            nc.sync.dma_start(sdl[:], hl_ap[:, :, SB2 + H:SB2 + 2 * H])

            # ---- phase C: per dst block (numer/denom), tail batched ----
            nma = wpool.tile([128, BPC, H, C + 1], bf16, tag="nma")
            for b in range(BPC):
                D = Dpad[b]
                g1 = g1f[:].rearrange("p (d w) -> p d w", w=RWB)[:, 0:D, :]
                nc.gpsimd.dma_gather(
                    g1[:, :, :], hf[L][:, :],
                    idx_sb[:, doff[b] * 8:(doff[b] + D) * 8],
                    D * 128, dreg[D], RWB, single_packet=False)
                g1s = g1.bitcast(bf16)  # [128, D, RWB//2]
                p = wpool.tile([128, D, H], f32, tag="p")
                sc = wpool.tile([128, D, H], f32, tag="sc")
                nc.vector.tensor_tensor(
                    sc[:], g1s[:, :, SB2:SB2 + H],
                    sdl[:, b:b + 1, :].broadcast_to([128, D, H]), ALU.add)
                lr = wpool.tile([128, D, H], f32, tag="lr")
                nc.vector.scalar_tensor_tensor(lr[:], sc[:], 0.2, sc[:],
                                               op0=ALU.mult, op1=ALU.max)
                nc.scalar.activation(p[:], lr[:], ACT.Exp)

                msg = msgf[:, 0:H * (C + 1) * D].rearrange(
                    "p (h c d) -> p h c d", h=H, c=C + 1)
                g1v = g1[:, :, 0:FW].rearrange("p d (h c) -> p h c d", h=H)
                pv = p[:].rearrange("p d h -> p h d").unsqueeze(2).broadcast_to(
                    [128, H, C + 1, D])
                nc.vector.tensor_tensor(msg, g1v, pv, ALU.mult)
                with nc.allow_low_precision("bf16 numer accumulate"):
                    nc.vector.tensor_reduce(nma[:, b, :, :], msg, axis=AX.X,
                                            op=ALU.add)

            # ---- batched tail over all BPC blocks ----
            dna = wpool.tile([128, BPC, H], f32, tag="dna")
            nc.vector.tensor_scalar(dna[:], nma[:, :, :, C], float(H),
                                    1e-16 * H, op0=ALU.mult, op1=ALU.add)
            rca = wpool.tile([128, BPC, H], f32, tag="rca")
            nc.vector.reciprocal(rca[:], dna[:])
            g1_f32 = g1f[:].bitcast(f32)
            if L < NL - 1:
                nm2a = msgf[:, 0:BPC * H * C].rearrange(
                    "p (b h c) -> p b h c", b=BPC, h=H)
                nc.vector.tensor_tensor(
                    nm2a, nma[:, :, :, 0:C],
                    rca[:].unsqueeze(3).broadcast_to([128, BPC, H, C]),
                    ALU.mult)
                xoa = g1_f32[:, 0:BPC * C].rearrange("p (b c) -> p b c", b=BPC)
                nc.vector.tensor_reduce(xoa,
                                        nm2a.rearrange("p b h c -> p b c h"),
                                        axis=AX.X, op=ALU.add)
                xba = g1_f32[:, BPC * C:2 * BPC * C].rearrange(
                    "p (b c) -> p b c", b=BPC)
                nc.vector.tensor_tensor(
                    xba, xoa,
                    bb_sb[L][:].unsqueeze(1).broadcast_to([128, BPC, C]),
                    ALU.add)
                x2ba = g1f[:].bitcast(bf16)[:, 4 * BPC * C:5 * BPC * C].rearrange(
                    "p (b c) -> p b c", b=BPC)
                nc.vector.tensor_scalar(x2ba, xba, 0.0, None, op0=ALU.max)
                nc.sync.dma_start(
                    x2d[L][:, :].rearrange("(b p) c -> p b c", p=128), x2ba)
            else:
                o3a = g1_f32[:, 0:BPC * C].rearrange("p (b c) -> p b c", b=BPC)
                nc.vector.tensor_tensor(
                    o3a, nma[:, :, 0, 0:C],
                    rca[:, :, 0:1].broadcast_to([128, BPC, C]), ALU.mult)
                o3s = wpool.tile([128, C], f32, tag="o3s")
                nc.vector.tensor_reduce(o3s[:],
                                        o3a.rearrange("p b c -> p c b"),
                                        axis=AX.X, op=ALU.add)
                nc.tensor.matmul(pfin[:], ones_sb[:], o3s[:],
                                 start=True, stop=True)

        fs = wpool.tile([1, C], f32, tag="fs")
        nc.scalar.copy(fs[:], pfin[:])
        nc.sync.dma_start(out_d[:, :], fs[:])

    nc.compile()
    return nc


# ----------------------------------------------------------------------------
# Entry points
# ----------------------------------------------------------------------------

_PREP_CACHE = {}


def make_cfg_and_maps(inputs):
    x = np.asarray(inputs["x"])
    edge_index = np.asarray(inputs["edge_index"])
    N, F_IN = x.shape
    NPAD = ((N + 1023) // 1024) * 1024
    Ws = [np.asarray(inputs[f"W{i}"]) for i in (1, 2, 3)]
    a_srcs = [np.asarray(inputs[f"as{i}"]) for i in (1, 2, 3)]
    a_dsts = [np.asarray(inputs[f"ad{i}"]) for i in (1, 2, 3)]
    bs = [np.asarray(inputs[f"b{i}"]) for i in (1, 2, 3)]
    HS = tuple(W.shape[0] for W in Ws)
    C = Ws[0].shape[2]

    pkey = (id(inputs["edge_index"]), edge_index.shape, N)
    if pkey not in _PREP_CACHE:
        _PREP_CACHE.clear()
        _PREP_CACHE[pkey] = prep_static(edge_index, N, NPAD)
    Dpad, idx_cores, node_of_row, PAD_P0 = _PREP_CACHE[pkey]
    xT, W_augs = prep_values(x, Ws, a_srcs, a_dsts, NPAD, node_of_row)

    cfg = dict(N=N, NPAD=NPAD, F_IN=F_IN, C=C, Dpad=Dpad, HS=HS,
               PAD_P0=PAD_P0)
    b3 = bs[2].astype(np.float32).reshape(1, C)
    NPC = NPAD // NCORES
    in_maps = []
    for c in range(NCORES):
        m = {
            "xT": np.ascontiguousarray(xT[:, c * NPC:(c + 1) * NPC]),
            "idxs": idx_cores[c],
        }
        for i in range(3):
            m[f"w{i+1}"] = W_augs[i]
        for i in range(2):
            m[f"bb{i+1}"] = np.ascontiguousarray(
                np.broadcast_to(bs[i].astype(np.float32)[None, :], (128, C)))
        in_maps.append(m)
    return cfg, in_maps, b3


_NC_CACHE = {}


def _get_nc(cfg, repeat=1):
    key = (repeat,) + tuple(sorted((k, v) for k, v in cfg.items()))
    if key not in _NC_CACHE:
        _NC_CACHE[key] = build_nc(cfg, repeat=repeat)
    return _NC_CACHE[key]


def run(inputs, trace=False, repeat=1, **kw):
    from concourse.bass_utils import run_bass_kernel_spmd
    cfg, in_maps, b3 = make_cfg_and_maps(inputs)
    nc = _get_nc(cfg, repeat=repeat)
    res = run_bass_kernel_spmd(nc, in_maps, core_ids=list(range(NCORES)),
                               trace=trace, **kw)
    out = np.zeros((1, cfg["C"]), dtype=np.float32)
    for r in res.results:
        out += r["out"]
    out = out * (1.0 / cfg["N"]) + b3
    return out, res


def kernel(**inputs) -> np.ndarray:
    out, _ = run(inputs)
    return out


# revision 31
# speedup vs baseline: 1.7043x; 1.7043x over previous
"""GAT (3-layer, PyG GATConv-style) Trainium2 Bass kernel, 8-core SPMD.

Strategy (degree-bucketed dst-major fixed-degree layout):
  - Nodes are permuted by in-degree (desc) and assigned to (core, block,
    partition): chunk b of 1024 sorted nodes -> block b on every core.
    Per-block slot count D_b = max in-degree within the chunk (padded to a
    multiple of 8), so padding waste stays ~15%.
  - Per layer: each core computes h_aug = x @ W_aug for its 2560 nodes
    (W_aug fuses per-head a_src/a_dst projections as trailing columns),
    stores to DRAM, one AllGather -> full node table hf.
  - Phase C per block of 128 dst nodes: dma_gather pulls the D_b incident
    src rows per dst into [128 dst, D_b, RW] (slot-major index tables), then
    a handful of giant DVE ops do the whole block: p = exp(leakyrelu(
    s_src + s_dst)), numer = reduce_d(p * h), denom = reduce_d(p),
    out = head_mean(numer / denom).  Padding slots point at a poisoned row
    (s_src = -1e9 -> p = 0), so no masking is needed.
  - Layer boundary: out blocks stored node-major to DRAM; the next layer's
    transposed activations are re-loaded via dma_gather(transpose=True)
    with an identity index table.
  - Layer 3 ends with a ones-vector matmul accumulating the node-sum
    partial; host sums the 8 per-core [1,128] partials.
"""

import numpy as np
import ml_dtypes

BF16 = ml_dtypes.bfloat16
NCORES = 8
GC = 6  # slots per gather chunk (6*128 = 768 idxs = 48 desc/engine)


# ----------------------------------------------------------------------------
# Host-side preprocessing
# ----------------------------------------------------------------------------

def _wrap16(idx_flat):
    """dma_gather index layout: [128, n/16] int16, idx i at [i%16, i//16],
    replicated across the 8 groups of 16 partitions."""
    n = idx_flat.shape[0]
    assert n % 16 == 0
    w = idx_flat.reshape(n // 16, 16).T.astype(np.int16)  # [16, n/16]
    return np.tile(w, (8, 1))  # [128, n/16]


def _wcols(H, C):
    """useful h_aug columns: H*(C+1) features+ones + 2H scores."""
    return H * (C + 1) + 2 * H


def _row_bytes(H, C):
    """gathered row bytes: H*(C+1) fp8 features+ones, then 2H bf16 scores at
    the next even byte, padded to a 256B multiple (dma_gather constraint)."""
    sbo = ((H * (C + 1) + 1) // 2) * 2
    used = sbo + 4 * H
    return ((used + 255) // 256) * 256


def _wpad(H, C):
    """W_aug padded column count (psum tile geometry: 512+128 or 256)."""
    return 640 if _wcols(H, C) > 512 else 256


def prep_static(edge_index, N, NPAD):
    """Degree-sorted node permutation + slot-major gather tables.

    Returns (Dpad, idx_cores, node_of_row, PAD_P0)."""
    loops = np.arange(N, dtype=np.int64)
    src = np.concatenate([edge_index[0].astype(np.int64), loops])
    dst = np.concatenate([edge_index[1].astype(np.int64), loops])
    deg = np.bincount(dst, minlength=NPAD)  # pad nodes have degree 0
    order = np.argsort(-deg, kind="stable")

    BPC = NPAD // (128 * NCORES)
    NPC = NPAD // NCORES
    node_of_row = np.empty(NPAD, dtype=np.int64)
    for b in range(BPC - 1):
        chunk = order[b * 1024:(b + 1) * 1024]
        q = np.arange(1024)
        rows = (q // 128) * NPC + b * 128 + (q % 128)
        node_of_row[rows] = chunk
    # last chunk: reals first on every core, pads fill the tail partitions
    last = order[(BPC - 1) * 1024:]
    n_real = int((deg[last] > 0).sum())
    assert n_real % NCORES == 0
    reals, pads = last[:n_real], last[n_real:]
    rpc = n_real // NCORES
    ppc = (1024 - n_real) // NCORES
    b = BPC - 1
    for c in range(NCORES):
        base = c * NPC + b * 128
        node_of_row[base:base + rpc] = reals[c * rpc:(c + 1) * rpc]
        node_of_row[base + rpc:base + 128] = pads[c * ppc:(c + 1) * ppc]
    row_of_node = np.empty(NPAD, dtype=np.int64)
    row_of_node[node_of_row] = np.arange(NPAD)
    PAD_P0 = rpc
    PADROW = NPAD  # dedicated poison row appended past the node table

    Dpad = []
    for b in range(BPC):
        mx = int(deg[order[b * 1024]])
        Dpad.append(max(1, mx))
    # group consecutive blocks (Dpad is non-increasing) into gather units:
    # unit slots k*Du bounded so the unit fits the g1 SBUF buffer
    KMAX = 88
    units = []
    i = 0
    while i < BPC:
        Du = Dpad[i]
        k = 1
        while i + k < BPC and (k + 1) * Du <= KMAX:
            k += 1
        units.append((i, k, Du))
        i += k

    sidx = np.argsort(dst, kind="stable")
    src_s, dst_s = src[sidx], dst[sidx]
    starts = np.searchsorted(dst_s, np.arange(NPAD))
    ends = np.searchsorted(dst_s, np.arange(NPAD) + 1)

    idx_cores = []
    for c in range(NCORES):
        cols = []
        for (b0, k, Du) in units:
            flat = np.full(k * Du * 128, PADROW, dtype=np.int64)
            for j in range(k):
                for p in range(128):
                    v = node_of_row[c * NPC + (b0 + j) * 128 + p]
                    s0, s1 = starts[v], ends[v]
                    n_e = s1 - s0
                    if n_e:
                        flat[(j * Du + np.arange(n_e)) * 128 + p] = \
                            row_of_node[src_s[s0:s1]]
            cols.append(_wrap16(flat))
        idx_cores.append(np.ascontiguousarray(np.concatenate(cols, axis=1)))
    return tuple(units), idx_cores, node_of_row, PAD_P0


def prep_values(x, Ws, a_srcs, a_dsts, NPAD, node_of_row):
    N, F = x.shape
    xp = np.zeros((NPAD, F), dtype=np.float32)
    xp[:N] = x
    xperm = xp[node_of_row]  # row r holds node node_of_row[r]
    xT = np.ascontiguousarray(xperm.T).astype(BF16)  # [F, NPAD]

    W_augs = []
    for W, a_s, a_d in zip(Ws, a_srcs, a_dsts):
        H, Fin, C = W.shape
        FW = H * (C + 1)
        wsrc = np.einsum("hfc,hc->fh", W, a_s)
        wdst = np.einsum("hfc,hc->fh", W, a_d)
        Wa = np.zeros((Fin, _wpad(H, C)), dtype=np.float32)
        for h in range(H):
            # col h*(C+1)+C stays 0: the ones column, memset on device
            Wa[:, h * (C + 1):h * (C + 1) + C] = W[h].reshape(Fin, C)
        Wa[:, FW:FW + H] = wsrc
        Wa[:, FW + H:FW + 2 * H] = wdst
        W_augs.append(Wa.astype(BF16))
    return xT, W_augs


# ----------------------------------------------------------------------------
# Device program
# ----------------------------------------------------------------------------

def build_nc(cfg, repeat=1):
    import concourse.bacc as bacc
    import concourse.mybir as mybir
    import concourse.tile as tile
    from contextlib import ExitStack

    f32 = mybir.dt.float32
    bf16 = mybir.dt.bfloat16
    f8 = mybir.dt.float8e4
    i16 = mybir.dt.int16
    ALU = mybir.AluOpType
    ACT = mybir.ActivationFunctionType
    AX = mybir.AxisListType

    N = cfg["N"]
    NPAD = cfg["NPAD"]
    F_IN = cfg["F_IN"]
    C = cfg["C"]
    UNITS = cfg["UNITS"]
    HS = cfg["HS"]
    PAD_P0 = cfg["PAD_P0"]
    BPC = NPAD // (128 * NCORES)
    NPC = NPAD // NCORES
    NL = len(HS)
    RWBs = [_row_bytes(H, C) for H in HS]
    WPs = [_wpad(H, C) for H in HS]
    FINs = [F_IN] + [C] * (NL - 1)
    DSUM = sum(k * Du for (_, k, Du) in UNITS)
    doff = [0]
    for (_, k, Du) in UNITS:
        doff.append(doff[-1] + k * Du)
    Dmax = max(Du for (_, _, Du) in UNITS)
    SMAX = max(k * Du for (_, k, Du) in UNITS)
    G1W = SMAX * max(RWBs)

    nc = bacc.Bacc("TRN2", target_bir_lowering=False, debug=False,
                   num_devices=NCORES)

    xT_d = nc.dram_tensor("xT", [F_IN, NPC], bf16, kind="ExternalInput")
    idx_d = nc.dram_tensor("idxs", [128, DSUM * 8], i16, kind="ExternalInput")
    W_d = [nc.dram_tensor(f"w{i+1}", [FINs[i], WPs[i]], bf16,
                          kind="ExternalInput") for i in range(NL)]
    bb_d = [nc.dram_tensor(f"bb{i+1}", [128, C], f32, kind="ExternalInput")
            for i in range(NL - 1)]
    out_d = nc.dram_tensor("out", [1, C], f32, kind="ExternalOutput")

    with tile.TileContext(nc, num_cores=NCORES) as tc, ExitStack() as ctx:
        dram = ctx.enter_context(tc.tile_pool(name="dram", bufs=1, space="DRAM"))
        cpool = ctx.enter_context(tc.tile_pool(name="consts", bufs=1))
        hpool = ctx.enter_context(tc.tile_pool(name="hs", bufs=1))
        wpool = ctx.enter_context(tc.tile_pool(name="work", bufs=1))
        psum = ctx.enter_context(tc.tile_pool(name="ps", bufs=2, space="PSUM"))

        hl = [dram.tile([NPC, RWBs[i]], f8, tag=f"hl{i}", name=f"hl{i}")
              for i in range(NL)]
        # one extra row past the node table: the poison row pad slots point at
        hf = [dram.tile([NPAD + 1, RWBs[i]], f8, tag=f"hf{i}", name=f"hf{i}")
              for i in range(NL)]
        x2d = [dram.tile([NPC, C], bf16, tag=f"x2d{i}", name=f"x2d{i}")
               for i in range(NL - 1)]

        xT_sb = cpool.tile([F_IN, NPC], bf16, tag="xT")
        nc.sync.dma_start(xT_sb[:], xT_d[:, :])
        idx_sb = cpool.tile([128, DSUM * 8], i16, tag="idx")
        nc.sync.dma_start(idx_sb[:], idx_d[:, :])
        W_sb = []
        for i in range(NL):
            w = cpool.tile([FINs[i], WPs[i]], bf16, tag=f"w{i}", name=f"w{i}")
            nc.sync.dma_start(w[:], W_d[i][:, :])
            W_sb.append(w)
        bb_sb = []
        for i in range(NL - 1):
            t = cpool.tile([128, C], f32, tag=f"bb{i}", name=f"bb{i}")
            nc.sync.dma_start(t[:], bb_d[i][:, :])
            bb_sb.append(t)
        ones_sb = cpool.tile([128, 1], f32, tag="ones")
        nc.vector.memset(ones_sb[:], 1.0)
        pois_sb = cpool.tile([1, max(RWBs)], f8, tag="pois")
        nc.vector.memset(pois_sb[:].bitcast(bf16), -1e9)
        x2T = [cpool.tile([C, NPC], bf16, tag=f"x2T{i}", name=f"x2T{i}")
               for i in range(NL - 1)]
        g1f = cpool.tile([128, G1W], f8, tag="g1f")
        hs4c = cpool.tile([128, 4, max(RWBs)], f8, tag="hs4c")
        nc.vector.memset(hs4c[:], 0.0)
        msgf = cpool.tile([128, max(HS) * (C + 1) * Dmax], bf16, tag="msgf")

        pfin = psum.tile([1, C], f32, tag="pfin", bufs=1)
        dreg = {s: nc.gpsimd.to_reg(s * 128) for s in sorted(set(k * Du for (_, k, Du) in UNITS))}

        for _rep in range(repeat):
         for L in range(NL):
            H = HS[L]
            RWB = RWBs[L]
            FW = H * (C + 1)
            SBO = ((FW + 1) // 2) * 2      # scores byte offset
            SB2 = SBO // 2                 # ... in bf16 elems
            WCOL = FW + 2 * H

            # ---- phase A: h_aug for own nodes ----
            if L > 0:
                # transposed activations via DMA XBAR transpose
                nc.sync.dma_start(x2T[L - 1][:, :], x2d[L - 1][:, :],
                                  transpose=True)
            lsrc = xT_sb if L == 0 else x2T[L - 1]
            WP = WPs[L]
            for g0 in range(0, BPC, 4):
                hs4 = hs4c[:, :, 0:RWB]
                hs4s = hs4.bitcast(bf16)  # [128, 4, RWB//2]
                if WP > 512:
                    pA4 = psum.tile([128, 4, 512], f32, tag="pA4", bufs=1)
                    pB4 = psum.tile([128, 4, WP - 512], f32, tag="pB4", bufs=1)
                else:
                    pA4 = psum.tile([128, 4, WP], f32, tag="pA4", bufs=1)
                for j in range(4):
                    lhsT = lsrc[:, (g0 + j) * 128:(g0 + j + 1) * 128]
                    if WP > 512:
                        nc.tensor.matmul(pA4[:, j, :], lhsT, W_sb[L][:, 0:512],
                                         start=True, stop=True)
                        nc.tensor.matmul(pB4[:, j, :], lhsT, W_sb[L][:, 512:WP],
                                         start=True, stop=True)
                    else:
                        nc.tensor.matmul(pA4[:, j, :], lhsT, W_sb[L][:, 0:WP],
                                         start=True, stop=True)
                if WP > 512:
                    nc.scalar.copy(hs4[:, :, 0:512], pA4[:])
                    nc.vector.tensor_copy(hs4[:, :, 512:FW],
                                          pB4[:, :, 0:FW - 512])
                    nc.vector.tensor_copy(hs4s[:, :, SB2:SB2 + 2 * H],
                                          pB4[:, :, FW - 512:WCOL - 512])
                else:
                    nc.scalar.copy(hs4[:, :, 0:FW], pA4[:, :, 0:FW])
                    nc.vector.tensor_copy(hs4s[:, :, SB2:SB2 + 2 * H],
                                          pA4[:, :, FW:WCOL])
                ones_ap = hs4[:, :, 0:FW].rearrange(
                    "p j (h c) -> p j h c", c=C + 1)[:, :, :, C]
                nc.vector.memset(ones_ap, 1.0)
                dst_ap = hl[L][g0 * 128:(g0 + 4) * 128, :].rearrange(
                    "(j p) w -> p j w", j=4)
                nc.sync.dma_start(dst_ap, hs4)

            # ---- phase B: allgather + poison row ----
            nc.gpsimd.collective_compute(
                "AllGather", mybir.AluOpType.bypass,
                replica_groups=[list(range(NCORES))],
                ins=[hl[L][:, :].opt()],
                outs=[hf[L][0:NPAD, :].opt()],
            )
            nc.sync.dma_start(hf[L][NPAD:NPAD + 1, :], pois_sb[:, 0:RWB])
            # own nodes' s_dst, [128, BPC, H]
            sdl = wpool.tile([128, BPC, H], bf16, tag="sdl")
            hl_ap = hl[L][:, :].bitcast(bf16).rearrange(
                "(b p) w -> p b w", p=128)
            nc.sync.dma_start(sdl[:], hl_ap[:, :, SB2 + H:SB2 + 2 * H])

            # ---- phase C: per unit gather/scores, per block numer ----
            nma = wpool.tile([128, BPC, H, C + 1], bf16, tag="nma")
            for ui, (b0, k, Du) in enumerate(UNITS):
                S = k * Du
                g1 = g1f[:].rearrange("p (j d w) -> p j d w", w=RWB,
                                      d=Dmax)[:, 0:k, 0:Du, :] \
                    if False else \
                    g1f[:].rearrange("p (s w) -> p s w", w=RWB)[:, 0:S, :] \
                    .rearrange("p (j d) w -> p j d w", d=Du)
                nc.gpsimd.dma_gather(
                    g1.rearrange("p j d w -> p (j d) w"), hf[L][:, :],
                    idx_sb[:, doff[ui] * 8:(doff[ui] + S) * 8],
                    S * 128, dreg[S], RWB, single_packet=False)
                g1s = g1.bitcast(bf16)  # [128, k, Du, RWB//2]
                p = wpool.tile([128, k, Du, H], f32, tag="p")
                sc = wpool.tile([128, k, Du, H], f32, tag="sc")
                nc.vector.tensor_tensor(
                    sc[:], g1s[:, :, :, SB2:SB2 + H],
                    sdl[:, b0:b0 + k, :].unsqueeze(2).broadcast_to(
                        [128, k, Du, H]), ALU.add)
                lr = wpool.tile([128, k, Du, H], f32, tag="lr")
                nc.vector.scalar_tensor_tensor(lr[:], sc[:], 0.2, sc[:],
                                               op0=ALU.mult, op1=ALU.max)
                nc.scalar.activation(p[:], lr[:], ACT.Exp)

                for j in range(k):
                    msg = msgf[:, 0:H * (C + 1) * Du].rearrange(
                        "p (h c d) -> p h c d", h=H, c=C + 1)
                    g1v = g1[:, j, :, 0:FW].rearrange(
                        "p d (h c) -> p h c d", h=H)
                    pv = p[:, j].rearrange("p d h -> p h d").unsqueeze(
                        2).broadcast_to([128, H, C + 1, Du])
                    nc.vector.tensor_tensor(msg, g1v, pv, ALU.mult)
                    with nc.allow_low_precision("bf16 numer accumulate"):
                        nc.vector.tensor_reduce(nma[:, b0 + j, :, :], msg,
                                                axis=AX.X, op=ALU.add)

            # ---- batched tail over all BPC blocks ----
            dna = wpool.tile([128, BPC, H], f32, tag="dna")
            nc.vector.tensor_scalar(dna[:], nma[:, :, :, C], float(H),
                                    1e-16 * H, op0=ALU.mult, op1=ALU.add)
            rca = wpool.tile([128, BPC, H], f32, tag="rca")
            nc.vector.reciprocal(rca[:], dna[:])
            g1_f32 = g1f[:].bitcast(f32)
            if L < NL - 1:
                nm2a = msgf[:, 0:BPC * H * C].rearrange(
                    "p (b h c) -> p b h c", b=BPC, h=H)
                nc.vector.tensor_tensor(
                    nm2a, nma[:, :, :, 0:C],
                    rca[:].unsqueeze(3).broadcast_to([128, BPC, H, C]),
                    ALU.mult)
                xoa = g1_f32[:, 0:BPC * C].rearrange("p (b c) -> p b c", b=BPC)
                nc.vector.tensor_reduce(xoa,
                                        nm2a.rearrange("p b h c -> p b c h"),
                                        axis=AX.X, op=ALU.add)
                xba = g1_f32[:, BPC * C:2 * BPC * C].rearrange(
                    "p (b c) -> p b c", b=BPC)
                nc.vector.tensor_tensor(
                    xba, xoa,
                    bb_sb[L][:].unsqueeze(1).broadcast_to([128, BPC, C]),
                    ALU.add)
                x2ba = g1f[:].bitcast(bf16)[:, 4 * BPC * C:5 * BPC * C].rearrange(
                    "p (b c) -> p b c", b=BPC)
                nc.vector.tensor_scalar(x2ba, xba, 0.0, None, op0=ALU.max)
                nc.sync.dma_start(
                    x2d[L][:, :].rearrange("(b p) c -> p b c", p=128), x2ba)
            else:
                o3a = g1_f32[:, 0:BPC * C].rearrange("p (b c) -> p b c", b=BPC)
                nc.vector.tensor_tensor(
                    o3a, nma[:, :, 0, 0:C],
                    rca[:, :, 0:1].broadcast_to([128, BPC, C]), ALU.mult)
                o3s = wpool.tile([128, C], f32, tag="o3s")
                nc.vector.tensor_reduce(o3s[:],
                                        o3a.rearrange("p b c -> p c b"),
                                        axis=AX.X, op=ALU.add)
                nc.tensor.matmul(pfin[:], ones_sb[:], o3s[:],
                                 start=True, stop=True)

        fs = wpool.tile([1, C], f32, tag="fs")
        nc.scalar.copy(fs[:], pfin[:])
        nc.sync.dma_start(out_d[:, :], fs[:])

    nc.compile()
    return nc


# ----------------------------------------------------------------------------
# Entry points
# ----------------------------------------------------------------------------

_PREP_CACHE = {}


def make_cfg_and_maps(inputs):
    x = np.asarray(inputs["x"])
    edge_index = np.asarray(inputs["edge_index"])
    N, F_IN = x.shape
    NPAD = ((N + 1023) // 1024) * 1024
    Ws = [np.asarray(inputs[f"W{i}"]) for i in (1, 2, 3)]
    a_srcs = [np.asarray(inputs[f"as{i}"]) for i in (1, 2, 3)]
    a_dsts = [np.asarray(inputs[f"ad{i}"]) for i in (1, 2, 3)]
    bs = [np.asarray(inputs[f"b{i}"]) for i in (1, 2, 3)]
    HS = tuple(W.shape[0] for W in Ws)
    C = Ws[0].shape[2]

    pkey = (id(inputs["edge_index"]), edge_index.shape, N)
    if pkey not in _PREP_CACHE:
        _PREP_CACHE.clear()
        _PREP_CACHE[pkey] = prep_static(edge_index, N, NPAD)
    units, idx_cores, node_of_row, PAD_P0 = _PREP_CACHE[pkey]
    xT, W_augs = prep_values(x, Ws, a_srcs, a_dsts, NPAD, node_of_row)

    cfg = dict(N=N, NPAD=NPAD, F_IN=F_IN, C=C, UNITS=units, HS=HS,
               PAD_P0=PAD_P0)
    b3 = bs[2].astype(np.float32).reshape(1, C)
    NPC = NPAD // NCORES
    in_maps = []
    for c in range(NCORES):
        m = {
            "xT": np.ascontiguousarray(xT[:, c * NPC:(c + 1) * NPC]),
            "idxs": idx_cores[c],
        }
        for i in range(3):
            m[f"w{i+1}"] = W_augs[i]
        for i in range(2):
            m[f"bb{i+1}"] = np.ascontiguousarray(
                np.broadcast_to(bs[i].astype(np.float32)[None, :], (128, C)))
        in_maps.append(m)
    return cfg, in_maps, b3


_NC_CACHE = {}


def _get_nc(cfg, repeat=1):
    key = (repeat,) + tuple(sorted((k, v) for k, v in cfg.items()))
    if key not in _NC_CACHE:
        _NC_CACHE[key] = build_nc(cfg, repeat=repeat)
    return _NC_CACHE[key]


def run(inputs, trace=False, repeat=1, **kw):
    from concourse.bass_utils import run_bass_kernel_spmd
    cfg, in_maps, b3 = make_cfg_and_maps(inputs)
    nc = _get_nc(cfg, repeat=repeat)
    res = run_bass_kernel_spmd(nc, in_maps, core_ids=list(range(NCORES)),
                               trace=trace, **kw)
    out = np.zeros((1, cfg["C"]), dtype=np.float32)
    for r in res.results:
        out += r["out"]
    out = out * (1.0 / cfg["N"]) + b3
    return out, res


def kernel(**inputs) -> np.ndarray:
    out, _ = run(inputs)
    return out
